# revision 47
# baseline (speedup 1.0000x reference)
"""Trainium2 Bass kernel for nn_Attention_5153960755626.

Multi-head attention (B=1, N=4096, C=768, H=12, D=64) distributed over 8
NeuronCores, sequence-parallel: core i computes attention output rows
[i*512, (i+1)*512).  Full K / V are exchanged with AllGather collectives.

v2 design (vs the hi/lo-bf16 baseline):
  - q, k in fp16: a single fp16 logits matmul reaches ~the same accuracy
    as the baseline's two-pass bf16 hi/lo split (fp16 has 10 mantissa
    bits vs bf16's 7) at half the TensorE cost and half the k-gather
    bytes.  exp output stays bf16 (logits span [-55, +47], fp16 would
    overflow at e^46).
  - k/q/v computed via fp32r matmuls from xT [C, n-slice]; W_q columns
    pre-scaled by sqrt(D) to fold the reference's q/scale quirk.
  - AllGathers emitted interleaved (k0,v0,k1,v1,...) right after phase-1
    compute, each gated only on its own shard DMA, so the CC stream
    starts at the all-core barrier (~20us) instead of ~80us and delivers
    each head-pair's k/v ahead of consumption.
  - V natural [n, d'] bf16 with a ones column per head so the PV matmul
    also produces the softmax denominator.
  - softmax normalization: reciprocal_approx_fast (1 DVE op, ~18 bits)
    + DMA partition-broadcast, replacing 12x 3.3us full-precision
    reciprocals + DRAM round-trips.
  - projection uses outT [c', n] directly as lhsT (bf16).

Pipelining: logits of group g+1 are emitted before PV of group g so the
TensorE stream never stalls waiting for ScalarE's exp.
"""

import os
import sys

sys.path.insert(0, "/opt/trn_rl_repo")

import numpy as np
import ml_dtypes

from contextlib import ExitStack

from concourse import bass, bacc, tile, mybir
from concourse.bass_utils import run_bass_kernel_spmd

NCORES = 8
N = 4096          # sequence length
C = 768           # channels
H = 12            # heads
DH = 64           # head dim
NL = N // NCORES  # local sequence rows per core (512)
CCH = C // 128    # channel chunks (6)
MC = N // 128     # key chunks over full sequence (32)
VW = 65           # per-head V width incl. ones column
SCALE = float(DH) ** 0.5  # reference divides q by D**-0.5 => q * 8

f32 = mybir.dt.float32
f32r = mybir.dt.float32r
f16 = mybir.dt.float16
bf16 = mybir.dt.bfloat16
Exp = mybir.ActivationFunctionType.Exp
MUL = mybir.AluOpType.mult
ADD = mybir.AluOpType.add


def _build_program():
    nc = bacc.Bacc(
        "TRN2",
        target_bir_lowering=False,
        debug=False,
        enable_asserts=False,
        num_devices=NCORES,
    )

    xT_d = nc.dram_tensor("xT", [C, NL], f32r, kind="ExternalInput").ap()
    qw_d = nc.dram_tensor("qkv_wT", [C, 3 * C], f32r, kind="ExternalInput").ap()
    bqk_d = nc.dram_tensor("qkv_b_qk", [128, 2 * CCH], f32, kind="ExternalInput").ap()
    bv_d = nc.dram_tensor("qkv_b_v", [1, C], f32, kind="ExternalInput").ap()
    pw_d = nc.dram_tensor("proj_wT", [C, C], bf16, kind="ExternalInput").ap()
    pb_d = nc.dram_tensor("proj_b", [1, C], f32, kind="ExternalInput").ap()
    out_d = nc.dram_tensor("out", [NL, C], f32, kind="ExternalOutput").ap()

    dbg = bool(int(os.environ.get("KDEBUG", "0")))
    if dbg:
        dbg_q = nc.dram_tensor("dbg_q", [2, 64, NL], f16, kind="ExternalOutput").ap()
        dbg_kall = nc.dram_tensor("dbg_kall", [NCORES, 2, 64, NL], f16, kind="ExternalOutput").ap()
        dbg_vall = nc.dram_tensor("dbg_vall", [NCORES, 128, (NL // 128) * 2 * VW], bf16, kind="ExternalOutput").ap()
        dbg_et = nc.dram_tensor("dbg_et", [128, NL], bf16, kind="ExternalOutput").ap()
        dbg_et1 = nc.dram_tensor("dbg_et1", [128, NL], bf16, kind="ExternalOutput").ap()
        dbg_pv = nc.dram_tensor("dbg_pv", [VW, NL], f32, kind="ExternalOutput").ap()
        dbg_outT = nc.dram_tensor("dbg_outT", [128, NL], bf16, kind="ExternalOutput").ap()

    groups = [list(range(NCORES))]

    with tile.TileContext(nc) as tc, ExitStack() as es:
        persist = es.enter_context(tc.tile_pool(name="persist", bufs=1))
        dram = es.enter_context(tc.tile_pool(name="dram", bufs=1, space="DRAM"))
        # program-lifetime pools for phase-1 tiles whose LAST op is a DMA
        # read (vloc -> vshard, kh/qh -> kshard/qf16): freeing them would let
        # the attention pool reuse their SBUF range, and cross-DMA-queue
        # ordering is not guaranteed on hardware (race seen on hw + sim)
        sc1 = es.enter_context(tc.tile_pool(name="sc1", bufs=2))
        kv1 = es.enter_context(tc.tile_pool(name="kv1", bufs=1))

        # ---- persistent SBUF ----
        # qf: per head, q in fp16 on partitions 0-63 (logits rhs)
        qf16 = [persist.tile([64, NL], f16, tag=f"qf16_{h}", name=f"qf16_{h}") for h in range(H)]

        outT = [persist.tile([128, NL], bf16, tag=f"outT{m}", name=f"outT{m}") for m in range(CCH)]
        bqk = persist.tile([128, 2 * CCH], f32, tag="bqk", name="bqk")
        vbc = persist.tile([128, C], f32, tag="vbc", name="vbc")
        pbc = persist.tile([128, C], f32, tag="pbc", name="pbc")
        projw = [persist.tile([128, C], bf16, tag=f"projw{m}", name=f"projw{m}") for m in range(CCH)]

        # ---- collective buffers (per head-pair so the AllGathers pipeline
        # under the attention loop instead of serializing in front of it) ----
        # k shards grouped TWO head-pairs per AllGather: the ~9us fixed cost
        # per collective op made the 12-op stream the steady-state pacer
        kshard = [dram.tile([4, 64, NL], f16, tag=f"kshard{u}", name=f"kshard{u}") for u in range(CCH // 2)]
        kall = [
            dram.tile([NCORES, 4, 64, NL], f16, tag=f"kall{u}", name=f"kall{u}", addr_space="Shared")
            for u in range(CCH // 2)
        ]
        # v shard layout [128, j*2VW] (j inner on the free dim) gives the
        # gather-load a single long-line DMA per pair
        vshard = [
            dram.tile([128, (NL // 128) * 2 * VW], bf16, tag=f"vshard{t}", name=f"vshard{t}")
            for t in range(CCH)
        ]
        vall = [
            dram.tile(
                [NCORES, 128, (NL // 128) * 2 * VW], bf16, tag=f"vall{t}", name=f"vall{t}", addr_space="Shared"
            )
            for t in range(CCH)
        ]

        recd = [dram.tile([1, NL], f32, tag=f"recd{i}", name=f"recd{i}") for i in range(2)]

        def allgather(src_t, dst_t):
            nc.gpsimd.collective_compute(
                "AllGather",
                mybir.AluOpType.bypass,
                replica_groups=groups,
                ins=[src_t.opt()],
                outs=[dst_t.opt()],
            )

        # ================= Phase 1: QKV projection =================
        with (
            tc.tile_pool(name="w1", bufs=1) as w1,
            tc.tile_pool(name="p1", bufs=3, space="PSUM") as p1,
        ):
            xts = [w1.tile([128, NL], f32r, tag=f"xts{c}", name=f"xts{c}") for c in range(CCH)]
            qwk = [w1.tile([128, C], f32r, tag=f"qwk{c}", name=f"qwk{c}") for c in range(CCH)]
            qwv = [w1.tile([128, C], f32r, tag=f"qwv{c}", name=f"qwv{c}") for c in range(CCH)]
            qwq = [w1.tile([128, C], f32r, tag=f"qwq{c}", name=f"qwq{c}") for c in range(CCH)]
            # load order: x + k-columns first (k chunk 0 gates the first
            # AllGather and the attention logits), then q, then v
            for c in range(CCH):
                nc.sync.dma_start(xts[c][:], xT_d[c * 128 : (c + 1) * 128, :])
                nc.sync.dma_start(qwk[c][:], qw_d[c * 128 : (c + 1) * 128, C : 2 * C])
            nc.sync.dma_start(bqk[:], bqk_d[:])
            for c in range(CCH):
                nc.sync.dma_start(qwq[c][:], qw_d[c * 128 : (c + 1) * 128, 0:C])
            for c in range(CCH):
                nc.sync.dma_start(qwv[c][:], qw_d[c * 128 : (c + 1) * 128, 2 * C : 3 * C])
            nc.sync.dma_start(vbc[:], bv_d[0:1, :].to_broadcast((128, C)))
            # proj weights loaded here (used only in phase 3) so their DMA
            # queue slots retire during phase-1 compute instead of colliding
            # with the attention gather-load burst
            nc.sync.dma_start(pbc[:], pb_d[0:1, :].to_broadcast((128, C)))
            for m in range(CCH):
                nc.sync.dma_start(projw[m][:], pw_d[m * 128 : (m + 1) * 128, :])

            def qk_psum(wtiles, m):
                ps = p1.tile([128, NL], f32, tag="p1qk", name="p1qk")
                for c in range(CCH):
                    nc.tensor.matmul(
                        ps[:],
                        lhsT=wtiles[c][:, m * 128 : (m + 1) * 128],
                        rhs=xts[c][:],
                        start=(c == 0),
                        stop=(c == CCH - 1),
                    )
                return ps

            # ---- k chunks first: their AllGathers gate attention logits
            for t in range(CCH):
                ps = qk_psum(qwk, t)
                kf = sc1.tile([128, NL], f32, tag="kf", name="kf")
                nc.vector.tensor_scalar_add(kf[:], ps[:], bqk[:, CCH + t : CCH + t + 1])
                kh = sc1.tile([128, NL], f16, tag="khs", name="khs")
                nc.vector.tensor_copy(kh[:], kf[:])
                for hh in range(2):
                    nc.sync.dma_start(
                        kshard[t // 2][2 * (t % 2) + hh], kh[hh * 64 : hh * 64 + 64, :]
                    )

            # ---- v natural layout [n, d'] bf16 with ones columns
            vloc = [kv1.tile([128, H * VW], bf16, tag=f"vloc{j}", name=f"vloc{j}") for j in range(NL // 128)]
            for j in range(NL // 128):
                nc.vector.memset(vloc[j][:], 1.0)
            for half in range(2):
                for j in range(NL // 128):
                    ps = p1.tile([128, 384], f32, tag="p1v", name="p1v")
                    for c in range(CCH):
                        nc.tensor.matmul(
                            ps[:],
                            lhsT=xts[c][:, j * 128 : (j + 1) * 128],
                            rhs=qwv[c][:, half * 384 : (half + 1) * 384],
                            start=(c == 0),
                            stop=(c == CCH - 1),
                        )
                    dst = vloc[j][:].rearrange("p (h e) -> p h e", e=VW)[
                        :, half * 6 : (half + 1) * 6, 0:DH
                    ]
                    vsrc_ = ps[:].rearrange("p (h e) -> p h e", e=DH)
                    bias = vbc[:, half * 384 : (half + 1) * 384].rearrange(
                        "p (h e) -> p h e", e=DH
                    )
                    nc.vector.tensor_tensor(dst, vsrc_, bias, ADD)
                for t in range(3 * half, 3 * half + 3):
                    for j in range(NL // 128):
                        nc.sync.dma_start(
                            vshard[t][:, j * 2 * VW : (j + 1) * 2 * VW],
                            vloc[j][:, 2 * t * VW : 2 * (t + 1) * VW],
                        )

            # ---- q chunks: fp16, head h moved onto partitions 0-63
            for t in range(CCH):
                ps = qk_psum(qwq, t)
                qf = sc1.tile([128, NL], f32, tag="qf", name="qf")
                nc.vector.tensor_scalar_add(qf[:], ps[:], bqk[:, t : t + 1])
                qh = sc1.tile([128, NL], f16, tag="qhs", name="qhs")
                nc.vector.tensor_copy(qh[:], qf[:])
                for hh in range(2):
                    nc.sync.dma_start(qf16[2 * t + hh][:], qh[hh * 64 : hh * 64 + 64, :])

            # interleaved gather emission: the cc stream executes in this
            # order; K for pairs 2u,2u+1 lands one step before their v's
            for u in range(CCH // 2):
                allgather(kshard[u], kall[u])
                allgather(vshard[2 * u], vall[2 * u])
                allgather(vshard[2 * u + 1], vall[2 * u + 1])

        # ================= Phase 2: attention =================
        with (
            tc.tile_pool(name="attn", bufs=2) as at,
            tc.tile_pool(name="lp", bufs=2, space="PSUM") as lpool,
            tc.tile_pool(name="pvp", bufs=2, space="PSUM") as pvpool,
            tc.tile_pool(name="ep", bufs=12) as epool,
            tc.tile_pool(name="np", bufs=2) as npool,
        ):
            GRPS = [3] * 10 + [2]  # 32 key-chunks per head

            # flat pipelined schedule: emit logits+exp of unit u, then PV of
            # unit u-1, so TensorE never stalls waiting for ScalarE's exp
            vp_tiles = {}
            pv_tiles = {}
            kp_tiles = {}

            loaded = set()

            def ensure_pair_loaded(t):
                # kp loads FIRST: the sync engine executes DMAs in program
                # order, and the k gather lands a step before the v gather
                if t in loaded or t >= CCH:
                    return
                loaded.add(t)
                for hh2 in range(2):
                    kp2 = at.tile([64, N], f16, tag="kp", name="kp", bufs=4)
                    for b in range(NCORES):
                        nc.sync.dma_start(
                            kp2[:, b * NL : (b + 1) * NL],
                            kall[t // 2][b, 2 * (t % 2) + hh2],
                        )
                    kp_tiles[2 * t + hh2] = kp2
                vp = at.tile([128, MC * 2 * VW], bf16, tag="vpair", name="vpair", bufs=3)
                JW = (NL // 128) * 2 * VW
                for b in range(NCORES):
                    nc.sync.dma_start(vp[:, b * JW : (b + 1) * JW], vall[t][b])
                vp_tiles[t] = vp

            def emit_logits(u):
                t, hh, gi, mc0, g = u
                h = 2 * t + hh
                if hh == 0 and gi == 0:
                    ensure_pair_loaded(t)
                if gi == 0:
                    pv_tiles[h] = pvpool.tile([VW, NL], f32, tag="pv", name="pv")
                kp = kp_tiles[h]
                lp = lpool.tile([128, 3 * NL], f32, tag="lg", name="lg")
                for j in range(g):
                    mc = mc0 + j
                    o = lp[:, j * NL : (j + 1) * NL]
                    nc.tensor.matmul(
                        o,
                        lhsT=kp[:, mc * 128 : (mc + 1) * 128],
                        rhs=qf16[h][:],
                        start=True,
                        stop=True,
                    )
                et = epool.tile([128, 3 * NL], bf16, tag="et", name="et")
                nc.scalar.activation(et[:, : g * NL], lp[:, : g * NL], Exp)
                if dbg and h == 0 and gi == 0:
                    nc.sync.dma_start(dbg_et[:], et[:, 0:NL])
                if dbg and h == 1 and gi == 0:
                    nc.sync.dma_start(dbg_et1[:], et[:, 0:NL])
                return et

            def emit_pv(u, et):
                t, hh, gi, mc0, g = u
                h = 2 * t + hh
                vp = vp_tiles[t]
                pv = pv_tiles[h]
                for j in range(g):
                    mc = mc0 + j
                    nc.tensor.matmul(
                        pv[:],
                        lhsT=vp[:, mc * 2 * VW + hh * VW : mc * 2 * VW + hh * VW + VW],
                        rhs=et[:, j * NL : (j + 1) * NL],
                        start=(mc == 0),
                        stop=(mc == MC - 1),
                    )
                if mc0 + g == MC:
                    # end of head: copy PSUM out right away (frees the pv
                    # bank for head h+2 without waiting on the DMA-heavy
                    # normalize chain), then normalize from the SBUF copy
                    # with a fast-approx reciprocal (~18 bits; the custom
                    # DVE op needs an SBUF partition-0 source) + broadcast.
                    pvs = npool.tile([VW, NL], f32, tag="pvs", name="pvs")
                    nc.vector.tensor_copy(pvs[:], pv[:])
                    if dbg and h == 0:
                        nc.sync.dma_start(dbg_pv[:], pvs[:])
                    den = npool.tile([1, NL], f32, tag="den", name="den")
                    nc.vector.tensor_copy(den[:], pvs[DH : DH + 1, :])
                    rec = npool.tile([1, NL], f32, tag="rec", name="rec")
                    nc.vector.reciprocal_approx_fast(rec[:], den[:])
                    rd = recd[h % 2]
                    nc.sync.dma_start(rd[:], rec[:])
                    rbc = npool.tile([64, NL], f32, tag="rbc", name="rbc")
                    nc.sync.dma_start(rbc[:], rd[0:1, :].to_broadcast((64, NL)))
                    nc.vector.tensor_tensor(
                        outT[t][hh * 64 : hh * 64 + 64, :], pvs[0:DH, :], rbc[:], MUL
                    )

            units = []
            for t in range(CCH):
                for hh in range(2):
                    mc0 = 0
                    for gi, g in enumerate(GRPS):
                        units.append((t, hh, gi, mc0, g))
                        mc0 += g

            prev = None
            for u in units:
                et = emit_logits(u)
                if prev is not None:
                    emit_pv(*prev)
                prev = (u, et)
            emit_pv(*prev)

            if dbg:
                nc.sync.dma_start(dbg_q[0], qf16[0][:])
                nc.sync.dma_start(dbg_q[1], qf16[1][:])
                nc.sync.dma_start(dbg_kall[:], kall[0][:, 0:2])
                nc.sync.dma_start(dbg_vall[:], vall[0][:])
                nc.sync.dma_start(dbg_outT[:], outT[0][:])

        # ================= Phase 3: projection =================
        with (
            tc.tile_pool(name="pp", bufs=2, space="PSUM") as ppool,
            tc.tile_pool(name="po", bufs=2) as opool,
        ):
            for j in range(NL // 128):
                osb = opool.tile([128, C], f32, tag="osb", name="osb")
                for half in range(2):
                    ps = ppool.tile([128, 384], f32, tag="pp", name="pp")
                    for m in range(CCH):
                        nc.tensor.matmul(
                            ps[:],
                            lhsT=outT[m][:, j * 128 : (j + 1) * 128],
                            rhs=projw[m][:, half * 384 : (half + 1) * 384],
                            start=(m == 0),
                            stop=(m == CCH - 1),
                        )
                    nc.vector.tensor_tensor(
                        osb[:, half * 384 : (half + 1) * 384],
                        ps[:],
                        pbc[:, half * 384 : (half + 1) * 384],
                        ADD,
                    )
                nc.sync.dma_start(out_d[j * 128 : (j + 1) * 128, :], osb[:])

    nc.compile()
    return nc


_PROGRAM = None


def _get_program():
    global _PROGRAM
    if _PROGRAM is None:
        _PROGRAM = _build_program()
    return _PROGRAM


def _round_fp32r(a):
    """Round fp32 to the fp32r bit format: 11-bit mantissa (RNE), low 12 bits zero."""
    u = np.ascontiguousarray(a, dtype=np.float32).view(np.uint32)
    lsb = (u >> 12) & 1
    u = (u + 0x7FF + lsb) & 0xFFFFF000
    return u.view(np.float32)


def _host_prep(x, qkv_w, qkv_b, proj_w, proj_b):
    x2 = np.asarray(x, dtype=np.float32).reshape(N, C)
    xT = _round_fp32r(np.ascontiguousarray(x2.T))  # [C, N]
    qkv_wT = np.ascontiguousarray(np.asarray(qkv_w, dtype=np.float32).T).copy()
    qkv_wT[:, :C] *= SCALE  # fold the q/scale quirk into W_q
    qkv_wT = _round_fp32r(qkv_wT)
    bqk = np.asarray(qkv_b, dtype=np.float32)[: 2 * C].reshape(2 * CCH, 128).T.copy()
    bqk[:, :CCH] *= SCALE  # fold scale into q bias too
    bv = np.asarray(qkv_b, dtype=np.float32)[2 * C :].reshape(1, C).copy()
    pwT = np.ascontiguousarray(np.asarray(proj_w, dtype=np.float32).T).astype(
        ml_dtypes.bfloat16
    )
    pb = np.asarray(proj_b, dtype=np.float32).reshape(1, C).copy()

    in_maps = []
    for i in range(NCORES):
        in_maps.append(
            {
                "xT": np.ascontiguousarray(xT[:, i * NL : (i + 1) * NL]),
                "qkv_wT": qkv_wT,
                "qkv_b_qk": bqk,
                "qkv_b_v": bv,
                "proj_wT": pwT,
                "proj_b": pb,
            }
        )
    return in_maps


def kernel(x, qkv_w, qkv_b, proj_w, proj_b):
    nc = _get_program()
    in_maps = _host_prep(x, qkv_w, qkv_b, proj_w, proj_b)
    kw = {}
    if os.environ.get("KERNEL_TRACE_DIR"):
        kw["tmpdir"] = os.environ["KERNEL_TRACE_DIR"]
    res = run_bass_kernel_spmd(
        nc,
        in_maps,
        core_ids=list(range(NCORES)),
        trace=bool(int(os.environ.get("KERNEL_TRACE", "0"))),
        **kw,
    )
    if res.exec_time_ns is not None:
        print(f"HW exec time: {res.exec_time_ns} ns", file=sys.stderr)
    out = np.concatenate(
        [np.asarray(res.results[i]["out"]) for i in range(NCORES)], axis=0
    )
    return out.reshape(1, N, C).astype(np.float32)


if __name__ == "__main__":
    rng = np.random.default_rng(0)
    x = rng.standard_normal((1, N, C), dtype=np.float32)
    qkv_w = (rng.standard_normal((3 * C, C)) * 0.01).astype(np.float32)
    qkv_b = np.zeros((3 * C,), np.float32)
    proj_w = (rng.standard_normal((C, C)) * 0.01).astype(np.float32)
    proj_b = np.zeros((C,), np.float32)
    out = kernel(x=x, qkv_w=qkv_w, qkv_b=qkv_b, proj_w=proj_w, proj_b=proj_b)
    print(out.shape, out.dtype)


# revision 54
# speedup vs baseline: 1.0612x; 1.0612x over previous
"""Trainium2 Bass kernel for nn_Attention_5153960755626.

Multi-head attention (B=1, N=4096, C=768, H=12, D=64) distributed over 8
NeuronCores, sequence-parallel: core i computes attention output rows
[i*512, (i+1)*512).  Full K / V are exchanged with AllGather collectives.

v2 design (vs the hi/lo-bf16 baseline):
  - q, k in fp16: a single fp16 logits matmul reaches ~the same accuracy
    as the baseline's two-pass bf16 hi/lo split (fp16 has 10 mantissa
    bits vs bf16's 7) at half the TensorE cost and half the k-gather
    bytes.  exp output stays bf16 (logits span [-55, +47], fp16 would
    overflow at e^46).
  - k/q/v computed via fp32r matmuls from xT [C, n-slice]; W_q columns
    pre-scaled by sqrt(D) to fold the reference's q/scale quirk.
  - AllGathers emitted interleaved (k0,v0,k1,v1,...) right after phase-1
    compute, each gated only on its own shard DMA, so the CC stream
    starts at the all-core barrier (~20us) instead of ~80us and delivers
    each head-pair's k/v ahead of consumption.
  - V natural [n, d'] bf16 with a ones column per head so the PV matmul
    also produces the softmax denominator.
  - softmax normalization: reciprocal_approx_fast (1 DVE op, ~18 bits)
    + DMA partition-broadcast, replacing 12x 3.3us full-precision
    reciprocals + DRAM round-trips.
  - projection uses outT [c', n] directly as lhsT (bf16).

Pipelining: logits of group g+1 are emitted before PV of group g so the
TensorE stream never stalls waiting for ScalarE's exp.
"""

import os
import sys

sys.path.insert(0, "/opt/trn_rl_repo")

import numpy as np
import ml_dtypes

from contextlib import ExitStack

from concourse import bass, bacc, tile, mybir
from concourse.bass_utils import run_bass_kernel_spmd

NCORES = 8
N = 4096          # sequence length
C = 768           # channels
H = 12            # heads
DH = 64           # head dim
NL = N // NCORES  # local sequence rows per core (512)
CCH = C // 128    # channel chunks (6)
MC = N // 128     # key chunks over full sequence (32)
VW = 65           # per-head V width incl. ones column
SCALE = float(DH) ** 0.5  # reference divides q by D**-0.5 => q * 8

f32 = mybir.dt.float32
f32r = mybir.dt.float32r
f16 = mybir.dt.float16
bf16 = mybir.dt.bfloat16
Exp = mybir.ActivationFunctionType.Exp
MUL = mybir.AluOpType.mult
ADD = mybir.AluOpType.add


def _build_program():
    nc = bacc.Bacc(
        "TRN2",
        target_bir_lowering=False,
        debug=False,
        enable_asserts=False,
        num_devices=NCORES,
    )

    xT_d = nc.dram_tensor("xT", [C, NL], f32r, kind="ExternalInput").ap()
    qw_d = nc.dram_tensor("qkv_wT", [C, 3 * C], f32r, kind="ExternalInput").ap()
    bqk_d = nc.dram_tensor("qkv_b_qk", [128, 2 * CCH], f32, kind="ExternalInput").ap()
    bv_d = nc.dram_tensor("qkv_b_v", [1, C], f32, kind="ExternalInput").ap()
    pw_d = nc.dram_tensor("proj_wT", [C, C], bf16, kind="ExternalInput").ap()
    pb_d = nc.dram_tensor("proj_b", [1, C], f32, kind="ExternalInput").ap()
    out_d = nc.dram_tensor("out", [NL, C], f32, kind="ExternalOutput").ap()

    dbg = bool(int(os.environ.get("KDEBUG", "0")))
    if dbg:
        dbg_q = nc.dram_tensor("dbg_q", [2, 64, NL], f16, kind="ExternalOutput").ap()
        dbg_kall = nc.dram_tensor("dbg_kall", [NCORES, 2, 64, NL], f16, kind="ExternalOutput").ap()
        dbg_vall = nc.dram_tensor("dbg_vall", [NCORES, 128, (NL // 128) * 2 * VW], bf16, kind="ExternalOutput").ap()
        dbg_et = nc.dram_tensor("dbg_et", [128, NL], bf16, kind="ExternalOutput").ap()
        dbg_et1 = nc.dram_tensor("dbg_et1", [128, NL], bf16, kind="ExternalOutput").ap()
        dbg_pv = nc.dram_tensor("dbg_pv", [VW, NL], f32, kind="ExternalOutput").ap()
        dbg_outT = nc.dram_tensor("dbg_outT", [128, NL], bf16, kind="ExternalOutput").ap()

    groups = [list(range(NCORES))]

    with tile.TileContext(nc) as tc, ExitStack() as es:
        persist = es.enter_context(tc.tile_pool(name="persist", bufs=1))
        dram = es.enter_context(tc.tile_pool(name="dram", bufs=1, space="DRAM"))
        # program-lifetime pools for phase-1 tiles whose LAST op is a DMA
        # read (vloc -> vshard, kh/qh -> kshard/qf16): freeing them would let
        # the attention pool reuse their SBUF range, and cross-DMA-queue
        # ordering is not guaranteed on hardware (race seen on hw + sim)
        sc1 = es.enter_context(tc.tile_pool(name="sc1", bufs=2))
        kv1 = es.enter_context(tc.tile_pool(name="kv1", bufs=1))

        # ---- persistent SBUF ----
        # qf: per head, q in fp16 on partitions 0-63 (logits rhs)
        qf16 = [persist.tile([64, NL], f16, tag=f"qf16_{h}", name=f"qf16_{h}") for h in range(H)]

        outT = [persist.tile([128, NL], bf16, tag=f"outT{m}", name=f"outT{m}") for m in range(CCH)]
        bqk = persist.tile([128, 2 * CCH], f32, tag="bqk", name="bqk")
        vbc = persist.tile([128, C], f32, tag="vbc", name="vbc")
        pbc = persist.tile([128, C], f32, tag="pbc", name="pbc")
        projw = [persist.tile([128, C], bf16, tag=f"projw{m}", name=f"projw{m}") for m in range(CCH)]

        # ---- collective buffers (per head-pair so the AllGathers pipeline
        # under the attention loop instead of serializing in front of it) ----
        kshard = [dram.tile([2, 64, NL], f16, tag=f"kshard{t}", name=f"kshard{t}") for t in range(CCH)]
        kall = [
            dram.tile([NCORES, 2, 64, NL], f16, tag=f"kall{t}", name=f"kall{t}", addr_space="Shared")
            for t in range(CCH)
        ]
        # v shard layout [128, j*2VW] (j inner on the free dim) gives the
        # gather-load a single long-line DMA per pair
        vshard = [
            dram.tile([128, (NL // 128) * 2 * VW], bf16, tag=f"vshard{t}", name=f"vshard{t}")
            for t in range(CCH)
        ]
        vall = [
            dram.tile(
                [NCORES, 128, (NL // 128) * 2 * VW], bf16, tag=f"vall{t}", name=f"vall{t}", addr_space="Shared"
            )
            for t in range(CCH)
        ]

        recd = [dram.tile([1, NL], f32, tag=f"recd{i}", name=f"recd{i}") for i in range(2)]

        def allgather(src_t, dst_t):
            nc.gpsimd.collective_compute(
                "AllGather",
                mybir.AluOpType.bypass,
                replica_groups=groups,
                ins=[src_t.opt()],
                outs=[dst_t.opt()],
            )

        # ================= Phase 1: QKV projection =================
        with (
            tc.tile_pool(name="w1", bufs=1) as w1,
            tc.tile_pool(name="p1", bufs=3, space="PSUM") as p1,
        ):
            xts = [w1.tile([128, NL], f32r, tag=f"xts{c}", name=f"xts{c}") for c in range(CCH)]
            qwk = [w1.tile([128, C], f32r, tag=f"qwk{c}", name=f"qwk{c}") for c in range(CCH)]
            qwv = [w1.tile([128, C], f32r, tag=f"qwv{c}", name=f"qwv{c}") for c in range(CCH)]
            qwq = [w1.tile([128, C], f32r, tag=f"qwq{c}", name=f"qwq{c}") for c in range(CCH)]
            # load order: x + k-columns first (k chunk 0 gates the first
            # AllGather and the attention logits), then q, then v
            for c in range(CCH):
                nc.sync.dma_start(xts[c][:], xT_d[c * 128 : (c + 1) * 128, :])
                nc.sync.dma_start(qwk[c][:], qw_d[c * 128 : (c + 1) * 128, C : 2 * C])
            nc.sync.dma_start(bqk[:], bqk_d[:])
            for c in range(CCH):
                nc.sync.dma_start(qwq[c][:], qw_d[c * 128 : (c + 1) * 128, 0:C])
            for c in range(CCH):
                nc.sync.dma_start(qwv[c][:], qw_d[c * 128 : (c + 1) * 128, 2 * C : 3 * C])
            nc.sync.dma_start(vbc[:], bv_d[0:1, :].to_broadcast((128, C)))
            # proj weights loaded here (used only in phase 3) so their DMA
            # queue slots retire during phase-1 compute instead of colliding
            # with the attention gather-load burst
            nc.sync.dma_start(pbc[:], pb_d[0:1, :].to_broadcast((128, C)))
            for m in range(CCH):
                nc.sync.dma_start(projw[m][:], pw_d[m * 128 : (m + 1) * 128, :])

            def qk_psum(wtiles, m):
                ps = p1.tile([128, NL], f32, tag="p1qk", name="p1qk")
                for c in range(CCH):
                    nc.tensor.matmul(
                        ps[:],
                        lhsT=wtiles[c][:, m * 128 : (m + 1) * 128],
                        rhs=xts[c][:],
                        start=(c == 0),
                        stop=(c == CCH - 1),
                    )
                return ps

            # ---- k chunks first: their AllGathers gate attention logits
            for t in range(CCH):
                ps = qk_psum(qwk, t)
                kf = sc1.tile([128, NL], f32, tag="kf", name="kf")
                nc.vector.tensor_scalar_add(kf[:], ps[:], bqk[:, CCH + t : CCH + t + 1])
                kh = sc1.tile([128, NL], f16, tag="khs", name="khs")
                nc.vector.tensor_copy(kh[:], kf[:])
                for hh in range(2):
                    nc.sync.dma_start(kshard[t][hh], kh[hh * 64 : hh * 64 + 64, :])

            # ---- v natural layout [n, d'] bf16 with ones columns
            vloc = [kv1.tile([128, H * VW], bf16, tag=f"vloc{j}", name=f"vloc{j}") for j in range(NL // 128)]
            for j in range(NL // 128):
                nc.vector.memset(vloc[j][:], 1.0)
            for half in range(2):
                for j in range(NL // 128):
                    ps = p1.tile([128, 384], f32, tag="p1v", name="p1v")
                    for c in range(CCH):
                        nc.tensor.matmul(
                            ps[:],
                            lhsT=xts[c][:, j * 128 : (j + 1) * 128],
                            rhs=qwv[c][:, half * 384 : (half + 1) * 384],
                            start=(c == 0),
                            stop=(c == CCH - 1),
                        )
                    dst = vloc[j][:].rearrange("p (h e) -> p h e", e=VW)[
                        :, half * 6 : (half + 1) * 6, 0:DH
                    ]
                    vsrc_ = ps[:].rearrange("p (h e) -> p h e", e=DH)
                    bias = vbc[:, half * 384 : (half + 1) * 384].rearrange(
                        "p (h e) -> p h e", e=DH
                    )
                    nc.vector.tensor_tensor(dst, vsrc_, bias, ADD)
                for t in range(3 * half, 3 * half + 3):
                    for j in range(NL // 128):
                        nc.sync.dma_start(
                            vshard[t][:, j * 2 * VW : (j + 1) * 2 * VW],
                            vloc[j][:, 2 * t * VW : 2 * (t + 1) * VW],
                        )

            # ---- q chunks: fp16, head h moved onto partitions 0-63
            for t in range(CCH):
                ps = qk_psum(qwq, t)
                qf = sc1.tile([128, NL], f32, tag="qf", name="qf")
                nc.vector.tensor_scalar_add(qf[:], ps[:], bqk[:, t : t + 1])
                qh = sc1.tile([128, NL], f16, tag="qhs", name="qhs")
                nc.vector.tensor_copy(qh[:], qf[:])
                for hh in range(2):
                    nc.sync.dma_start(qf16[2 * t + hh][:], qh[hh * 64 : hh * 64 + 64, :])

            # interleaved gather emission: the cc stream executes in this
            # order; logits of pair t need k_t one step before PV needs v_t
            for t in range(CCH):
                allgather(kshard[t], kall[t])
                allgather(vshard[t], vall[t])

        # ================= Phase 2: attention =================
        with (
            tc.tile_pool(name="attn", bufs=2) as at,
            tc.tile_pool(name="lp", bufs=2, space="PSUM") as lpool,
            tc.tile_pool(name="pvp", bufs=2, space="PSUM") as pvpool,
            tc.tile_pool(name="ep", bufs=6) as epool,
            tc.tile_pool(name="np", bufs=2) as npool,
        ):
            GRPS = [3] * 10 + [2]  # 32 key-chunks per head

            # flat pipelined schedule: emit logits+exp of unit u, then PV of
            # unit u-1, so TensorE never stalls waiting for ScalarE's exp
            vp_tiles = {}
            pv_tiles = {}
            kp_tiles = {}

            loaded = set()

            def ensure_pair_loaded(t):
                # kp loads FIRST: the sync engine executes DMAs in program
                # order, and the k gather lands a step before the v gather
                if t in loaded or t >= CCH:
                    return
                loaded.add(t)
                for hh2 in range(2):
                    kp2 = at.tile([64, N], f16, tag="kp", name="kp", bufs=4)
                    for b in range(NCORES):
                        nc.sync.dma_start(
                            kp2[:, b * NL : (b + 1) * NL], kall[t][b, hh2]
                        )
                    kp_tiles[2 * t + hh2] = kp2
                vp = at.tile([128, MC * 2 * VW], bf16, tag="vpair", name="vpair", bufs=3)
                JW = (NL // 128) * 2 * VW
                for b in range(NCORES):
                    nc.sync.dma_start(vp[:, b * JW : (b + 1) * JW], vall[t][b])
                vp_tiles[t] = vp

            def emit_logits(u):
                t, hh, gi, mc0, g = u
                h = 2 * t + hh
                if hh == 0 and gi == 0:
                    ensure_pair_loaded(t)
                if gi == 0:
                    pv_tiles[h] = pvpool.tile([VW, NL], f32, tag="pv", name="pv")
                kp = kp_tiles[h]
                lp = lpool.tile([128, 3 * NL], f32, tag="lg", name="lg")
                for j in range(g):
                    mc = mc0 + j
                    o = lp[:, j * NL : (j + 1) * NL]
                    nc.tensor.matmul(
                        o,
                        lhsT=kp[:, mc * 128 : (mc + 1) * 128],
                        rhs=qf16[h][:],
                        start=True,
                        stop=True,
                    )
                et = epool.tile([128, 3 * NL], bf16, tag="et", name="et")
                nc.scalar.activation(et[:, : g * NL], lp[:, : g * NL], Exp)
                if dbg and h == 0 and gi == 0:
                    nc.sync.dma_start(dbg_et[:], et[:, 0:NL])
                if dbg and h == 1 and gi == 0:
                    nc.sync.dma_start(dbg_et1[:], et[:, 0:NL])
                return et

            def emit_pv(u, et):
                t, hh, gi, mc0, g = u
                h = 2 * t + hh
                vp = vp_tiles[t]
                pv = pv_tiles[h]
                for j in range(g):
                    mc = mc0 + j
                    nc.tensor.matmul(
                        pv[:],
                        lhsT=vp[:, mc * 2 * VW + hh * VW : mc * 2 * VW + hh * VW + VW],
                        rhs=et[:, j * NL : (j + 1) * NL],
                        start=(mc == 0),
                        stop=(mc == MC - 1),
                    )
                if mc0 + g == MC:
                    if dbg and h == 0:
                        pvs = npool.tile([VW, NL], f32, tag="pvs", name="pvs")
                        nc.vector.tensor_copy(pvs[:], pv[:])
                        nc.sync.dma_start(dbg_pv[:], pvs[:])
                    # end of head: normalize with a fast-approx reciprocal
                    # (~18 bits, plenty for 1/denominator) + DMA broadcast.
                    # The custom-DVE approx op misreads PSUM/partition-64
                    # sources on hw, so stage the row to SBUF partition 0.
                    den = npool.tile([1, NL], f32, tag="den", name="den")
                    nc.vector.tensor_copy(den[:], pv[DH : DH + 1, :])
                    rec = npool.tile([1, NL], f32, tag="rec", name="rec")
                    nc.vector.reciprocal_approx_fast(rec[:], den[:])
                    rd = recd[h % 2]
                    nc.sync.dma_start(rd[:], rec[:])
                    rbc = npool.tile([64, NL], f32, tag="rbc", name="rbc")
                    nc.sync.dma_start(rbc[:], rd[0:1, :].to_broadcast((64, NL)))
                    nc.vector.tensor_tensor(
                        outT[t][hh * 64 : hh * 64 + 64, :], pv[0:DH, :], rbc[:], MUL
                    )

            units = []
            for t in range(CCH):
                for hh in range(2):
                    mc0 = 0
                    for gi, g in enumerate(GRPS):
                        units.append((t, hh, gi, mc0, g))
                        mc0 += g

            prev = None
            for u in units:
                et = emit_logits(u)
                if prev is not None:
                    emit_pv(*prev)
                prev = (u, et)
            emit_pv(*prev)

            if dbg:
                nc.sync.dma_start(dbg_q[0], qf16[0][:])
                nc.sync.dma_start(dbg_q[1], qf16[1][:])
                nc.sync.dma_start(dbg_kall[:], kall[0][:])
                nc.sync.dma_start(dbg_vall[:], vall[0][:])
                nc.sync.dma_start(dbg_outT[:], outT[0][:])

        # ================= Phase 3: projection =================
        with (
            tc.tile_pool(name="pp", bufs=2, space="PSUM") as ppool,
            tc.tile_pool(name="po", bufs=2) as opool,
        ):
            for j in range(NL // 128):
                osb = opool.tile([128, C], f32, tag="osb", name="osb")
                for half in range(2):
                    ps = ppool.tile([128, 384], f32, tag="pp", name="pp")
                    for m in range(CCH):
                        nc.tensor.matmul(
                            ps[:],
                            lhsT=outT[m][:, j * 128 : (j + 1) * 128],
                            rhs=projw[m][:, half * 384 : (half + 1) * 384],
                            start=(m == 0),
                            stop=(m == CCH - 1),
                        )
                    nc.vector.tensor_tensor(
                        osb[:, half * 384 : (half + 1) * 384],
                        ps[:],
                        pbc[:, half * 384 : (half + 1) * 384],
                        ADD,
                    )
                nc.sync.dma_start(out_d[j * 128 : (j + 1) * 128, :], osb[:])

    nc.compile()
    return nc


_PROGRAM = None


def _get_program():
    global _PROGRAM
    if _PROGRAM is None:
        _PROGRAM = _build_program()
    return _PROGRAM


def _round_fp32r(a):
    """Round fp32 to the fp32r bit format: 11-bit mantissa (RNE), low 12 bits zero."""
    u = np.ascontiguousarray(a, dtype=np.float32).view(np.uint32)
    lsb = (u >> 12) & 1
    u = (u + 0x7FF + lsb) & 0xFFFFF000
    return u.view(np.float32)


def _host_prep(x, qkv_w, qkv_b, proj_w, proj_b):
    x2 = np.asarray(x, dtype=np.float32).reshape(N, C)
    xT = _round_fp32r(np.ascontiguousarray(x2.T))  # [C, N]
    qkv_wT = np.ascontiguousarray(np.asarray(qkv_w, dtype=np.float32).T).copy()
    qkv_wT[:, :C] *= SCALE  # fold the q/scale quirk into W_q
    qkv_wT = _round_fp32r(qkv_wT)
    bqk = np.asarray(qkv_b, dtype=np.float32)[: 2 * C].reshape(2 * CCH, 128).T.copy()
    bqk[:, :CCH] *= SCALE  # fold scale into q bias too
    bv = np.asarray(qkv_b, dtype=np.float32)[2 * C :].reshape(1, C).copy()
    pwT = np.ascontiguousarray(np.asarray(proj_w, dtype=np.float32).T).astype(
        ml_dtypes.bfloat16
    )
    pb = np.asarray(proj_b, dtype=np.float32).reshape(1, C).copy()

    in_maps = []
    for i in range(NCORES):
        in_maps.append(
            {
                "xT": np.ascontiguousarray(xT[:, i * NL : (i + 1) * NL]),
                "qkv_wT": qkv_wT,
                "qkv_b_qk": bqk,
                "qkv_b_v": bv,
                "proj_wT": pwT,
                "proj_b": pb,
            }
        )
    return in_maps


def kernel(x, qkv_w, qkv_b, proj_w, proj_b):
    nc = _get_program()
    in_maps = _host_prep(x, qkv_w, qkv_b, proj_w, proj_b)
    kw = {}
    if os.environ.get("KERNEL_TRACE_DIR"):
        kw["tmpdir"] = os.environ["KERNEL_TRACE_DIR"]
    res = run_bass_kernel_spmd(
        nc,
        in_maps,
        core_ids=list(range(NCORES)),
        trace=bool(int(os.environ.get("KERNEL_TRACE", "0"))),
        **kw,
    )
    if res.exec_time_ns is not None:
        print(f"HW exec time: {res.exec_time_ns} ns", file=sys.stderr)
    out = np.concatenate(
        [np.asarray(res.results[i]["out"]) for i in range(NCORES)], axis=0
    )
    return out.reshape(1, N, C).astype(np.float32)


if __name__ == "__main__":
    rng = np.random.default_rng(0)
    x = rng.standard_normal((1, N, C), dtype=np.float32)
    qkv_w = (rng.standard_normal((3 * C, C)) * 0.01).astype(np.float32)
    qkv_b = np.zeros((3 * C,), np.float32)
    proj_w = (rng.standard_normal((C, C)) * 0.01).astype(np.float32)
    proj_b = np.zeros((C,), np.float32)
    out = kernel(x=x, qkv_w=qkv_w, qkv_b=qkv_b, proj_w=proj_w, proj_b=proj_b)
    print(out.shape, out.dtype)
